# revision 10
# baseline (speedup 1.0000x reference)
"""Trainium2 Bass kernel for nn_EnsemblePolicyHeads (MoE routing head).

Self-contained: accepts FULL inputs, shards batch across the 8 NeuronCores
(data parallel, weights replicated), returns the FULL [8192, 64] output.

Key layout trick: the z@W1 contraction over i (0..2047) is decomposed into
16 slices j with i = 16*p + j (p = SBUF partition). Under this "j-layout"
the W1 expert matrices load from HBM fully contiguously (8KB/partition, one
descriptor per partition -> full HBM rate; the classic (ko ki)->ki,ko
rearrange needs 512B descriptors and is descriptor-rate-bound ~150GB/s).
Only z needs an extra SBUF->SBUF hop: fp32->fp16 cast-DMA, XBAR transpose
into the m-layout (i = 128*m + p), then 16 cheap shuffle DMAs into j-layout.
Logits run on the m-layout zT directly.

Pipeline: PE warmed with junk matmuls from ~6.5us (HAM clock gate); z nt0
then W1 experts stream on the gpsimd SWDGE ring in FIFO order; z nt1 is
slotted into that FIFO behind W1[13]. ps_o is initialized by the b2 matmul
(start=True); finalize's PSUM read happens before the next nt's b2 write.
"""
import sys

for _p in ("/opt/trn_rl_repo",):
    if _p not in sys.path:
        sys.path.insert(0, _p)


import numpy as np
from contextlib import ExitStack

import concourse.bass as bass
import concourse.tile as tile
from concourse import bacc, mybir
from concourse.masks import make_identity
from concourse.tile_rust import add_dep_helper

F32 = mybir.dt.float32
BF16 = mybir.dt.float16  # fp16: same PE rate as bf16, 8x finer mantissa
AF = mybir.ActivationFunctionType
ALU = mybir.AluOpType

D = 2048      # input dim
H = 128       # hidden
O = 64        # output dim
E = 16        # num experts
P = 128
KO = D // P   # 16 k-slices (m-layout for logits, j-layout for W1)
NT_SIZE = 512
N_WARM = 40   # junk warmup matmuls to keep PE busy+warm during z load


def build_kernel(Bc: int):
    assert Bc % NT_SIZE == 0
    NT = Bc // NT_SIZE
    SUBS = NT_SIZE // P        # 128-row blocks per nt
    NBLK = Bc // P             # total 128-row blocks (8)

    nc = bacc.Bacc("TRN2", target_bir_lowering=False, debug=False)
    z_ap = nc.dram_tensor("z", [Bc, D], F32, kind="ExternalInput").ap()
    W1_ap = nc.dram_tensor("W1", [E, D, H], F32, kind="ExternalInput").ap()
    b1_ap = nc.dram_tensor("b1", [E, H], F32, kind="ExternalInput").ap()
    W2_ap = nc.dram_tensor("W2", [E, H, O], F32, kind="ExternalInput").ap()
    b2_ap = nc.dram_tensor("b2", [E, O], F32, kind="ExternalInput").ap()
    Wa_ap = nc.dram_tensor("Wa", [D, E], F32, kind="ExternalInput").ap()
    ba_ap = nc.dram_tensor("ba", [E], F32, kind="ExternalInput").ap()
    out_ap = nc.dram_tensor("out", [Bc, O], F32, kind="ExternalOutput").ap()

    with tile.TileContext(nc) as tc, ExitStack() as ctx:
        persist = ctx.enter_context(tc.tile_pool(name="persist", bufs=1))
        zc_pool = ctx.enter_context(tc.tile_pool(name="zc", bufs=3))
        t_pool = ctx.enter_context(tc.tile_pool(name="t", bufs=3))
        hm_pool = ctx.enter_context(tc.tile_pool(name="hm", bufs=3))
        res_pool = ctx.enter_context(tc.tile_pool(name="res", bufs=2))
        outsb_pool = ctx.enter_context(tc.tile_pool(name="outsb", bufs=2))
        psA = ctx.enter_context(tc.tile_pool(name="psA", bufs=3, space="PSUM"))
        psB = ctx.enter_context(tc.tile_pool(name="psB", bufs=2, space="PSUM"))
        psC = ctx.enter_context(tc.tile_pool(name="psC", bufs=1, space="PSUM"))
        psD = ctx.enter_context(tc.tile_pool(name="psD", bufs=2, space="PSUM"))

        # ---- persistent tiles ----
        zTm = persist.tile([P, KO, Bc], BF16)    # z[b, 128m+p]  (logits)
        zTj = persist.tile([P, KO, Bc], BF16)    # z[b, 16p+j]   (W1 matmuls)
        w1j = [persist.tile([P, KO, H], BF16, name=f"w1j{e}") for e in range(E)]
        Wasb = persist.tile([P, KO, E], F32)
        Wabf = persist.tile([P, KO, E], BF16)
        W2sb = persist.tile([P, E, O], F32)
        W2bf = persist.tile([P, E, O], BF16)
        b2sb = persist.tile([E, O], F32)
        b2bf = persist.tile([E, O], BF16)
        ba_sb = persist.tile([E, 1], F32)
        b1sb = persist.tile([E, H], F32)
        b1bf = persist.tile([E, H], BF16)
        b1T = persist.tile([P, E], F32)
        expT = persist.tile([E, Bc], BF16)
        attn_be = persist.tile([P, Bc // P, E], F32)
        denomT = persist.tile([P, Bc // P], F32)
        recipT = persist.tile([P, Bc // P], F32)
        id_bf = persist.tile([P, P], BF16)
        rep_sel = persist.tile([E, E, P], BF16)
        junk = persist.tile([P, NT_SIZE], BF16)

        # ================= emission (program order matters per engine) ======
        nc.vector.memset(junk, 0.0)
        make_identity(nc, id_bf)

        # sync ring first in trace order: small fp32 weights (the gpsimd
        # casts below read these, so the DMAs must be traced first)
        nc.sync.dma_start(ba_sb[:], ba_ap[:, None])
        nc.sync.dma_start(b1sb[:], b1_ap[:])
        nc.sync.dma_start(W2sb[:], W2_ap.rearrange("e h o -> h e o"))
        nc.sync.dma_start(b2sb[:], b2_ap[:])

        # gpsimd SWDGE FIFO: z nt0 casts, setup, W1 stream (z nt1 slotted in)
        zcs = {}

        def z_cast_dma(blk):
            zcs[blk] = zc_pool.tile([P, D], BF16, tag="zc", name=f"zc{blk}")
            for h2 in range(2):
                nc.gpsimd.dma_start(
                    zcs[blk][:, h2 * D // 2:(h2 + 1) * D // 2],
                    z_ap[blk * P:(blk + 1) * P, h2 * D // 2:(h2 + 1) * D // 2])

        for blk in range(SUBS):
            z_cast_dma(blk)
        nc.gpsimd.memset(rep_sel, 0.0)
        nc.gpsimd.affine_select(
            out=rep_sel, in_=rep_sel,
            compare_op=ALU.not_equal, fill=1.0, base=0,
            pattern=[[-1, E], [0, P]], channel_multiplier=1,
        )

        def w1_load(e):
            # fully contiguous: partition p <- rows 16p..16p+15 of W1[e]
            nc.gpsimd.dma_start(
                w1j[e][:], W1_ap[e].rearrange("(p j) h -> p (j h)", p=P))

        w1_load(0)
        w1_load(1)
        nc.gpsimd.tensor_copy(W2bf[:], W2sb[:])
        nc.gpsimd.tensor_copy(b2bf[:], b2sb[:])
        for e in range(2, 14):
            w1_load(e)
        for blk in range(SUBS, NBLK):   # z nt1, behind W1[13] in the FIFO
            z_cast_dma(blk)
        w1_load(14)
        w1_load(15)

        # sync ring continues: nt0 XBARs; out stores later.
        def xbar(blk):
            for h2 in range(2):
                nc.sync.dma_start_transpose(
                    zTm[:, h2 * KO // 2:(h2 + 1) * KO // 2, blk * P:(blk + 1) * P],
                    zcs[blk][:, h2 * D // 2:(h2 + 1) * D // 2])

        for blk in range(SUBS):
            xbar(blk)

        # scalar ring: Wa slices, then zTm->zTj shuffles per (m, nt)
        for ko in range(KO):
            nc.scalar.dma_start(Wasb[:, ko, :], Wa_ap[ko * P:(ko + 1) * P, :])

        def shuffle_z(a, nt):
            bs = slice(nt * NT_SIZE, (nt + 1) * NT_SIZE)
            # zTj[8a+pl, j, b] = zTm[16*pl + j, a, b]  (both sides flat-match)
            nc.scalar.dma_start(zTj[8 * a:8 * (a + 1), :, bs], zTm[:, a, bs])

        for a in range(KO):
            shuffle_z(a, 0)

        # DVE early casts
        nc.vector.tensor_copy(b1bf[:], b1sb[:])
        nc.vector.tensor_copy(Wabf[:], Wasb[:])

        # ---- PE stream: warmup junk, b1 transpose, logits, main loop ----
        for i in range(N_WARM):
            ps_j = psB.tile([P, NT_SIZE], F32, tag="ps_r", name="ps_warm")
            nc.tensor.matmul(ps_j[:], junk[:, :P], junk[:], start=True, stop=True)

        ps_b1 = psD.tile([P, E], BF16, tag="ps_tr")
        nc.tensor.transpose(ps_b1[:], b1bf[:], id_bf[:E, :E])
        nc.scalar.copy(b1T[:], ps_b1[:])

        def logits_nt(nt):
            bs = slice(nt * NT_SIZE, (nt + 1) * NT_SIZE)
            ps_l = psD.tile([E, NT_SIZE], F32, tag="ps_tr", name="ps_l")
            for ko in range(KO):
                nc.tensor.matmul(
                    ps_l[:], Wabf[:, ko, :], zTm[:, ko, bs],
                    start=(ko == 0), stop=(ko == KO - 1))
            nc.scalar.activation(expT[:, bs], ps_l[:], AF.Exp, bias=ba_sb[:])

        def denom_nt(nt):
            for sub in range(SUBS):
                blk = nt * SUBS + sub
                ps_t = psD.tile([P, E], BF16, tag="ps_tr")
                nc.tensor.transpose(
                    ps_t[:], expT[:, blk * P:(blk + 1) * P], id_bf[:E, :E])
                nc.scalar.copy(attn_be[:, blk, :], ps_t[:])
            nts = slice(nt * SUBS, (nt + 1) * SUBS)
            nc.vector.reduce_sum(
                denomT[:, nts, None], attn_be[:, nts, :], axis=mybir.AxisListType.X)
            nc.vector.reciprocal(recipT[:, nts], denomT[:, nts])

        def finalize_tail(nt, res):
            for sub in range(SUBS):
                blk = nt * SUBS + sub
                ps_t2 = psD.tile([P, O], BF16, tag="ps_tr")
                nc.tensor.transpose(
                    ps_t2[:], res[:, sub * P:(sub + 1) * P], id_bf[:O, :O])
                outsb = outsb_pool.tile([P, O], F32)
                nc.scalar.activation(outsb[:], ps_t2[:], AF.Copy,
                                     scale=recipT[:, blk:blk + 1])
                nc.sync.dma_start(out_ap[blk * P:(blk + 1) * P, :], outsb[:])

        logits_nt(0)

        # ---- main loop ----
        pend_w2 = []
        pend_fin = None

        def flush_w2(keep, stop=False):
            while len(pend_w2) > keep:
                pe_, phm, po = pend_w2.pop(0)
                nc.tensor.matmul(po[:], W2bf[:, pe_, :], phm[:],
                                 start=False, stop=(stop and not pend_w2))

        for nt in range(NT):
            bs = slice(nt * NT_SIZE, (nt + 1) * NT_SIZE)
            ps_o = psC.tile([O, NT_SIZE], F32)
            nc.tensor.matmul(ps_o[:], b2bf[:], expT[:, bs], start=True, stop=False)
            for e in range(E):
                ps_h = psA.tile([P, NT_SIZE], F32)
                for j in range(KO):
                    nc.tensor.matmul(
                        ps_h[:], w1j[e][:, j, :], zTj[:, j, bs],
                        start=(j == 0), stop=(j == KO - 1))
                ps_r = psB.tile([P, NT_SIZE], F32, tag="ps_r")
                nc.tensor.matmul(ps_r[:], rep_sel[:, e, :], expT[:, bs],
                                 start=True, stop=True)
                flush_w2(1)
                if e == 2:
                    denom_nt(nt)
                if e == 13 and nt + 1 < NT:
                    logits_nt(nt + 1)
                if pend_fin is not None and e == 1:
                    finalize_tail(*pend_fin)
                    pend_fin = None
                t = t_pool.tile([P, NT_SIZE], F32)
                nc.scalar.activation(t[:], ps_h[:], AF.Relu, bias=b1T[:, e:e + 1])
                hm = hm_pool.tile([P, NT_SIZE], BF16)
                nc.vector.tensor_tensor(hm[:], t[:], ps_r[:], ALU.mult)
                pend_w2.append((e, hm, ps_o))
                # nt1 z prep: XBARs then shuffles, all before logits(nt1)@e13
                if nt == 0 and 8 <= e < 8 + SUBS:
                    xbar(SUBS + (e - 8))
                if nt == 0 and e == 12:
                    for a in range(KO):
                        shuffle_z(a, 1)
            flush_w2(0, stop=True)
            # read ps_o now: the next nt's b2 matmul (start=True) reuses the
            # same PSUM bank and must come after this copy in program order
            res = res_pool.tile([O, NT_SIZE], BF16)
            nc.vector.tensor_copy(res[:], ps_o[:])
            pend_fin = (nt, res)
        finalize_tail(*pend_fin)

    nc.compile()
    return nc


def ref_numpy(z, W1, b1, W2, b2, Wa, ba):
    B = z.shape[0]
    z = z.reshape(B, -1).astype(np.float64)
    logits = z @ Wa.astype(np.float64) + ba
    a = np.exp(logits - logits.max(axis=1, keepdims=True))
    a /= a.sum(axis=1, keepdims=True)
    h = np.maximum(np.einsum("bi,eih->beh", z, W1.astype(np.float64)) + b1, 0)
    o = np.einsum("beh,eho->beo", h, W2.astype(np.float64)) + b2
    return np.einsum("be,beo->bo", a, o).astype(np.float32)


# ---------------------------------------------------------------------------
# Harness entry point
# ---------------------------------------------------------------------------
N_CORES = 8
B_TOTAL = 8192
BC = B_TOTAL // N_CORES

_nc_cache = {}


def _get_nc():
    if "nc" not in _nc_cache:
        _nc_cache["nc"] = build_kernel(BC)
    return _nc_cache["nc"]


def kernel(z_i, W1, b1, W2, b2, Wa, ba):
    from concourse.bass_utils import run_bass_kernel_spmd

    z = np.ascontiguousarray(np.asarray(z_i, dtype=np.float32).reshape(B_TOTAL, D))
    W1 = np.ascontiguousarray(np.asarray(W1, dtype=np.float32))
    b1 = np.ascontiguousarray(np.asarray(b1, dtype=np.float32))
    W2 = np.ascontiguousarray(np.asarray(W2, dtype=np.float32))
    b2 = np.ascontiguousarray(np.asarray(b2, dtype=np.float32))
    Wa = np.ascontiguousarray(np.asarray(Wa, dtype=np.float32))
    ba = np.ascontiguousarray(np.asarray(ba, dtype=np.float32))

    nc = _get_nc()
    in_maps = [
        dict(z=z[c * BC:(c + 1) * BC], W1=W1, b1=b1, W2=W2, b2=b2, Wa=Wa, ba=ba)
        for c in range(N_CORES)
    ]
    res = run_bass_kernel_spmd(nc, in_maps, core_ids=list(range(N_CORES)))
    return np.concatenate([res.results[c]["out"] for c in range(N_CORES)], axis=0)


# revision 13
# speedup vs baseline: 1.8031x; 1.8031x over previous
"""Trainium2 Bass kernel for nn_EnsemblePolicyHeads (MoE routing head).

Self-contained: accepts FULL inputs, shards batch across the 8 NeuronCores
(data parallel, weights replicated), returns the FULL [8192, 64] output.

Key layout trick: the z@W1 contraction over i (0..2047) is decomposed into
16 slices j with i = 16*p + j (p = SBUF partition). Under this "j-layout"
the W1 expert matrices load from HBM fully contiguously (8KB/partition, one
descriptor per partition -> full HBM rate; the classic (ko ki)->ki,ko
rearrange needs 512B descriptors and is descriptor-rate-bound ~150GB/s).
z is cast-DMA'd (fp32->fp16) contiguously and transposed into the j-layout
on the PE (strided column slices, 16 transposes per 128-row block). The XBAR
dma_start_transpose path is avoided entirely: each one acts as a ~5-6us
global DMA barrier (deadlock guard), which starves the weight stream.

Pipeline: PE warmed with junk matmuls from ~6.5us (HAM clock gate); z nt0
then W1 experts stream on the gpsimd SWDGE ring in FIFO order; z nt1 is
slotted into that FIFO behind W1[13]. ps_o is initialized by the b2 matmul
(start=True); finalize's PSUM read happens before the next nt's b2 write.
"""
import sys

for _p in ("/opt/trn_rl_repo",):
    if _p not in sys.path:
        sys.path.insert(0, _p)


import numpy as np
from contextlib import ExitStack

import concourse.bass as bass
import concourse.tile as tile
from concourse import bacc, mybir
from concourse.masks import make_identity
from concourse.tile_rust import add_dep_helper

F32 = mybir.dt.float32
BF16 = mybir.dt.float16  # fp16: same PE rate as bf16, 8x finer mantissa
AF = mybir.ActivationFunctionType
ALU = mybir.AluOpType

D = 2048      # input dim
H = 128       # hidden
O = 64        # output dim
E = 16        # num experts
P = 128
KO = D // P   # 16 k-slices (m-layout for logits, j-layout for W1)
NT_SIZE = 512
N_WARM = 40   # junk warmup matmuls to keep PE busy+warm during z load


def build_kernel(Bc: int):
    assert Bc % NT_SIZE == 0
    NT = Bc // NT_SIZE
    SUBS = NT_SIZE // P        # 128-row blocks per nt
    NBLK = Bc // P             # total 128-row blocks (8)

    nc = bacc.Bacc("TRN2", target_bir_lowering=False, debug=False)
    z_ap = nc.dram_tensor("z", [Bc, D], F32, kind="ExternalInput").ap()
    W1_ap = nc.dram_tensor("W1", [E, D, H], F32, kind="ExternalInput").ap()
    b1_ap = nc.dram_tensor("b1", [E, H], F32, kind="ExternalInput").ap()
    W2_ap = nc.dram_tensor("W2", [E, H, O], F32, kind="ExternalInput").ap()
    b2_ap = nc.dram_tensor("b2", [E, O], F32, kind="ExternalInput").ap()
    Wa_ap = nc.dram_tensor("Wa", [D, E], F32, kind="ExternalInput").ap()
    ba_ap = nc.dram_tensor("ba", [E], F32, kind="ExternalInput").ap()
    out_ap = nc.dram_tensor("out", [Bc, O], F32, kind="ExternalOutput").ap()

    with tile.TileContext(nc) as tc, ExitStack() as ctx:
        persist = ctx.enter_context(tc.tile_pool(name="persist", bufs=1))
        zc_pool = ctx.enter_context(tc.tile_pool(name="zc", bufs=3))
        t_pool = ctx.enter_context(tc.tile_pool(name="t", bufs=3))
        hm_pool = ctx.enter_context(tc.tile_pool(name="hm", bufs=3))
        res_pool = ctx.enter_context(tc.tile_pool(name="res", bufs=2))
        outsb_pool = ctx.enter_context(tc.tile_pool(name="outsb", bufs=2))
        psA = ctx.enter_context(tc.tile_pool(name="psA", bufs=3, space="PSUM"))
        psB = ctx.enter_context(tc.tile_pool(name="psB", bufs=2, space="PSUM"))
        psC = ctx.enter_context(tc.tile_pool(name="psC", bufs=1, space="PSUM"))
        psD = ctx.enter_context(tc.tile_pool(name="psD", bufs=2, space="PSUM"))

        # ---- persistent tiles ----
        zTj = persist.tile([P, KO, Bc], BF16)    # z[b, 16p+j]
        w1j = [persist.tile([P, KO, H], BF16, name=f"w1j{e}") for e in range(E)]
        Wabf = persist.tile([P, KO, E], BF16)    # Wa[16p+j, e]
        W2sb = persist.tile([P, E, O], F32)
        W2bf = persist.tile([P, E, O], BF16)
        b2sb = persist.tile([E, O], F32)
        b2bf = persist.tile([E, O], BF16)
        ba_sb = persist.tile([E, 1], F32)
        b1sb = persist.tile([E, H], F32)
        b1bf = persist.tile([E, H], BF16)
        b1T = persist.tile([P, E], F32)
        expT = persist.tile([E, Bc], BF16)
        attn_be = persist.tile([P, Bc // P, E], F32)
        denomT = persist.tile([P, Bc // P], F32)
        recipT = persist.tile([P, Bc // P], F32)
        id_bf = persist.tile([P, P], BF16)
        rep_sel = persist.tile([E, E, P], BF16)
        junk = persist.tile([P, NT_SIZE], BF16)

        # ================= emission (program order matters per engine) ======
        nc.vector.memset(junk, 0.0)
        make_identity(nc, id_bf)

        # sync ring first in trace order: small fp32 weights (the gpsimd
        # casts below read these, so the DMAs must be traced first)
        nc.sync.dma_start(ba_sb[:], ba_ap[:, None])
        nc.sync.dma_start(b1sb[:], b1_ap[:])
        nc.sync.dma_start(W2sb[:], W2_ap.rearrange("e h o -> h e o"))
        nc.sync.dma_start(b2sb[:], b2_ap[:])

        # gpsimd SWDGE FIFO: z nt0 casts, setup, W1 stream (z nt1 slotted in)
        zcs = {}

        def z_cast_dma(blk):
            zcs[blk] = zc_pool.tile([P, D], BF16, tag="zc", name=f"zc{blk}")
            for h2 in range(2):
                nc.gpsimd.dma_start(
                    zcs[blk][:, h2 * D // 2:(h2 + 1) * D // 2],
                    z_ap[blk * P:(blk + 1) * P, h2 * D // 2:(h2 + 1) * D // 2])

        for blk in range(SUBS):
            z_cast_dma(blk)
        nc.gpsimd.memset(rep_sel, 0.0)
        nc.gpsimd.affine_select(
            out=rep_sel, in_=rep_sel,
            compare_op=ALU.not_equal, fill=1.0, base=0,
            pattern=[[-1, E], [0, P]], channel_multiplier=1,
        )

        def w1_load(e):
            # fully contiguous: partition p <- rows 16p..16p+15 of W1[e]
            nc.gpsimd.dma_start(
                w1j[e][:], W1_ap[e].rearrange("(p j) h -> p (j h)", p=P))

        # Wa in the same j-layout, one contiguous cast-DMA (1KB/partition)
        nc.gpsimd.dma_start(
            Wabf[:], Wa_ap.rearrange("(p j) e -> p (j e)", p=P))
        w1_load(0)
        w1_load(1)
        nc.gpsimd.tensor_copy(W2bf[:], W2sb[:])
        nc.gpsimd.tensor_copy(b2bf[:], b2sb[:])
        for e in range(2, 10):
            w1_load(e)
        for blk in range(SUBS, NBLK):   # z nt1, behind W1[9] in the FIFO
            z_cast_dma(blk)
        for e in range(10, E):
            w1_load(e)

        # DVE early casts
        nc.vector.tensor_copy(b1bf[:], b1sb[:])

        # ---- PE stream ----
        def warm(n):
            for _ in range(n):
                ps_j = psB.tile([P, NT_SIZE], F32, tag="ps_r", name="ps_warm")
                nc.tensor.matmul(ps_j[:], junk[:, :P], junk[:],
                                 start=True, stop=True)

        def pe_transpose_block(blk):
            # zTj[p, j, blk cols] = z[b, 16p+j]: strided column slices
            zr = zcs[blk][:].rearrange("b (p j) -> b j p", j=KO)
            for jp in range(KO // 2):
                ps_t = psD.tile([P, 2 * P], BF16, tag="ps_tr", name="ps_ztr")
                for k in range(2):
                    nc.tensor.transpose(
                        ps_t[:, k * P:(k + 1) * P], zr[:, 2 * jp + k, :], id_bf[:])
                dst = zTj[:, 2 * jp:2 * jp + 2, blk * P:(blk + 1) * P]
                if jp % 2 == 0:
                    nc.scalar.copy(dst, ps_t[:].rearrange("p (k b) -> p k b", k=2))
                else:
                    nc.vector.tensor_copy(
                        dst, ps_t[:].rearrange("p (k b) -> p k b", k=2))

        warm(10)
        for blk in range(SUBS):
            pe_transpose_block(blk)
            warm(3)

        ps_b1 = psD.tile([P, E], BF16, tag="ps_tr")
        nc.tensor.transpose(ps_b1[:], b1bf[:], id_bf[:E, :E])
        nc.scalar.copy(b1T[:], ps_b1[:])

        def logits_nt(nt):
            bs = slice(nt * NT_SIZE, (nt + 1) * NT_SIZE)
            ps_l = psD.tile([E, NT_SIZE], F32, tag="ps_tr", name="ps_l")
            for ko in range(KO):
                nc.tensor.matmul(
                    ps_l[:], Wabf[:, ko, :], zTj[:, ko, bs],
                    start=(ko == 0), stop=(ko == KO - 1))
            nc.scalar.activation(expT[:, bs], ps_l[:], AF.Exp, bias=ba_sb[:])

        def denom_nt(nt):
            for sub in range(SUBS):
                blk = nt * SUBS + sub
                ps_t = psD.tile([P, E], BF16, tag="ps_tr")
                nc.tensor.transpose(
                    ps_t[:], expT[:, blk * P:(blk + 1) * P], id_bf[:E, :E])
                nc.scalar.copy(attn_be[:, blk, :], ps_t[:])
            nts = slice(nt * SUBS, (nt + 1) * SUBS)
            nc.vector.reduce_sum(
                denomT[:, nts, None], attn_be[:, nts, :], axis=mybir.AxisListType.X)
            nc.vector.reciprocal(recipT[:, nts], denomT[:, nts])

        def finalize_tail(nt, res):
            for sub in range(SUBS):
                blk = nt * SUBS + sub
                ps_t2 = psD.tile([P, O], BF16, tag="ps_tr")
                nc.tensor.transpose(
                    ps_t2[:], res[:, sub * P:(sub + 1) * P], id_bf[:O, :O])
                outsb = outsb_pool.tile([P, O], F32)
                nc.scalar.activation(outsb[:], ps_t2[:], AF.Copy,
                                     scale=recipT[:, blk:blk + 1])
                nc.sync.dma_start(out_ap[blk * P:(blk + 1) * P, :], outsb[:])

        logits_nt(0)

        # ---- main loop ----
        pend_w2 = []
        pend_fin = None

        def flush_w2(keep, stop=False):
            while len(pend_w2) > keep:
                pe_, phm, po = pend_w2.pop(0)
                nc.tensor.matmul(po[:], W2bf[:, pe_, :], phm[:],
                                 start=False, stop=(stop and not pend_w2))

        for nt in range(NT):
            bs = slice(nt * NT_SIZE, (nt + 1) * NT_SIZE)
            ps_o = psC.tile([O, NT_SIZE], F32)
            nc.tensor.matmul(ps_o[:], b2bf[:], expT[:, bs], start=True, stop=False)
            for e in range(E):
                ps_h = psA.tile([P, NT_SIZE], F32)
                for j in range(KO):
                    nc.tensor.matmul(
                        ps_h[:], w1j[e][:, j, :], zTj[:, j, bs],
                        start=(j == 0), stop=(j == KO - 1))
                ps_r = psB.tile([P, NT_SIZE], F32, tag="ps_r")
                nc.tensor.matmul(ps_r[:], rep_sel[:, e, :], expT[:, bs],
                                 start=True, stop=True)
                flush_w2(1)
                if e == 2:
                    denom_nt(nt)
                if e == 13 and nt + 1 < NT:
                    logits_nt(nt + 1)
                if pend_fin is not None and e == 1:
                    finalize_tail(*pend_fin)
                    pend_fin = None
                t = t_pool.tile([P, NT_SIZE], F32)
                nc.scalar.activation(t[:], ps_h[:], AF.Relu, bias=b1T[:, e:e + 1])
                hm = hm_pool.tile([P, NT_SIZE], BF16)
                nc.vector.tensor_tensor(hm[:], t[:], ps_r[:], ALU.mult)
                pend_w2.append((e, hm, ps_o))
                # nt1 z prep: PE transposes, all before logits(nt1)@e13
                if nt == 0 and 8 <= e < 8 + SUBS:
                    pe_transpose_block(SUBS + (e - 8))
            flush_w2(0, stop=True)
            # read ps_o now: the next nt's b2 matmul (start=True) reuses the
            # same PSUM bank and must come after this copy in program order
            res = res_pool.tile([O, NT_SIZE], BF16)
            nc.vector.tensor_copy(res[:], ps_o[:])
            pend_fin = (nt, res)
        finalize_tail(*pend_fin)

    nc.compile()
    return nc


def ref_numpy(z, W1, b1, W2, b2, Wa, ba):
    B = z.shape[0]
    z = z.reshape(B, -1).astype(np.float64)
    logits = z @ Wa.astype(np.float64) + ba
    a = np.exp(logits - logits.max(axis=1, keepdims=True))
    a /= a.sum(axis=1, keepdims=True)
    h = np.maximum(np.einsum("bi,eih->beh", z, W1.astype(np.float64)) + b1, 0)
    o = np.einsum("beh,eho->beo", h, W2.astype(np.float64)) + b2
    return np.einsum("be,beo->bo", a, o).astype(np.float32)


# ---------------------------------------------------------------------------
# Harness entry point
# ---------------------------------------------------------------------------
N_CORES = 8
B_TOTAL = 8192
BC = B_TOTAL // N_CORES

_nc_cache = {}


def _get_nc():
    if "nc" not in _nc_cache:
        _nc_cache["nc"] = build_kernel(BC)
    return _nc_cache["nc"]


def kernel(z_i, W1, b1, W2, b2, Wa, ba):
    from concourse.bass_utils import run_bass_kernel_spmd

    z = np.ascontiguousarray(np.asarray(z_i, dtype=np.float32).reshape(B_TOTAL, D))
    W1 = np.ascontiguousarray(np.asarray(W1, dtype=np.float32))
    b1 = np.ascontiguousarray(np.asarray(b1, dtype=np.float32))
    W2 = np.ascontiguousarray(np.asarray(W2, dtype=np.float32))
    b2 = np.ascontiguousarray(np.asarray(b2, dtype=np.float32))
    Wa = np.ascontiguousarray(np.asarray(Wa, dtype=np.float32))
    ba = np.ascontiguousarray(np.asarray(ba, dtype=np.float32))

    nc = _get_nc()
    in_maps = [
        dict(z=z[c * BC:(c + 1) * BC], W1=W1, b1=b1, W2=W2, b2=b2, Wa=Wa, ba=ba)
        for c in range(N_CORES)
    ]
    res = run_bass_kernel_spmd(nc, in_maps, core_ids=list(range(N_CORES)))
    return np.concatenate([res.results[c]["out"] for c in range(N_CORES)], axis=0)


# revision 14
# speedup vs baseline: 1.8186x; 1.0086x over previous
"""Trainium2 Bass kernel for nn_EnsemblePolicyHeads (MoE routing head).

Self-contained: accepts FULL inputs, shards batch across the 8 NeuronCores
(data parallel, weights replicated), returns the FULL [8192, 64] output.

Key layout trick: the z@W1 contraction over i (0..2047) is decomposed into
16 slices j with i = 16*p + j (p = SBUF partition). Under this "j-layout"
the W1 expert matrices load from HBM fully contiguously (8KB/partition, one
descriptor per partition -> full HBM rate; the classic (ko ki)->ki,ko
rearrange needs 512B descriptors and is descriptor-rate-bound ~150GB/s).
z is cast-DMA'd (fp32->fp16) contiguously and transposed into the j-layout
on the PE (strided column slices, 16 transposes per 128-row block). The XBAR
dma_start_transpose path is avoided entirely: each one acts as a ~5-6us
global DMA barrier (deadlock guard), which starves the weight stream.

Pipeline: PE warmed with junk matmuls from ~6.5us (HAM clock gate); z nt0
then W1 experts stream on the gpsimd SWDGE ring in FIFO order; z nt1 is
slotted into that FIFO behind W1[13]. ps_o is initialized by the b2 matmul
(start=True); finalize's PSUM read happens before the next nt's b2 write.
"""
import sys

for _p in ("/opt/trn_rl_repo",):
    if _p not in sys.path:
        sys.path.insert(0, _p)


import numpy as np
from contextlib import ExitStack

import concourse.bass as bass
import concourse.tile as tile
from concourse import bacc, mybir
from concourse.masks import make_identity
from concourse.tile_rust import add_dep_helper

F32 = mybir.dt.float32
BF16 = mybir.dt.float16  # fp16: same PE rate as bf16, 8x finer mantissa
AF = mybir.ActivationFunctionType
ALU = mybir.AluOpType

D = 2048      # input dim
H = 128       # hidden
O = 64        # output dim
E = 16        # num experts
P = 128
KO = D // P   # 16 k-slices (m-layout for logits, j-layout for W1)
NT_SIZE = 512
N_WARM = 40   # junk warmup matmuls to keep PE busy+warm during z load


def build_kernel(Bc: int):
    assert Bc % NT_SIZE == 0
    NT = Bc // NT_SIZE
    SUBS = NT_SIZE // P        # 128-row blocks per nt
    NBLK = Bc // P             # total 128-row blocks (8)

    nc = bacc.Bacc("TRN2", target_bir_lowering=False, debug=False)
    z_ap = nc.dram_tensor("z", [Bc, D], F32, kind="ExternalInput").ap()
    W1_ap = nc.dram_tensor("W1", [E, D, H], F32, kind="ExternalInput").ap()
    b1_ap = nc.dram_tensor("b1", [E, H], F32, kind="ExternalInput").ap()
    W2_ap = nc.dram_tensor("W2", [E, H, O], F32, kind="ExternalInput").ap()
    b2_ap = nc.dram_tensor("b2", [E, O], F32, kind="ExternalInput").ap()
    Wa_ap = nc.dram_tensor("Wa", [D, E], F32, kind="ExternalInput").ap()
    ba_ap = nc.dram_tensor("ba", [E], F32, kind="ExternalInput").ap()
    out_ap = nc.dram_tensor("out", [Bc, O], F32, kind="ExternalOutput").ap()

    with tile.TileContext(nc) as tc, ExitStack() as ctx:
        persist = ctx.enter_context(tc.tile_pool(name="persist", bufs=1))
        zc_pool = ctx.enter_context(tc.tile_pool(name="zc", bufs=3))
        t_pool = ctx.enter_context(tc.tile_pool(name="t", bufs=3))
        hm_pool = ctx.enter_context(tc.tile_pool(name="hm", bufs=3))
        res_pool = ctx.enter_context(tc.tile_pool(name="res", bufs=2))
        outsb_pool = ctx.enter_context(tc.tile_pool(name="outsb", bufs=2))
        psA = ctx.enter_context(tc.tile_pool(name="psA", bufs=3, space="PSUM"))
        psB = ctx.enter_context(tc.tile_pool(name="psB", bufs=2, space="PSUM"))
        psC = ctx.enter_context(tc.tile_pool(name="psC", bufs=1, space="PSUM"))
        psD = ctx.enter_context(tc.tile_pool(name="psD", bufs=2, space="PSUM"))

        # ---- persistent tiles ----
        zTj = persist.tile([P, KO, Bc], BF16)    # z[b, 16p+j]
        w1j = [persist.tile([P, KO, H], BF16, name=f"w1j{e}") for e in range(E)]
        Wabf = persist.tile([P, KO, E], BF16)    # Wa[16p+j, e]
        W2sb = persist.tile([P, E, O], F32)
        W2bf = persist.tile([P, E, O], BF16)
        b2sb = persist.tile([E, O], F32)
        b2bf = persist.tile([E, O], BF16)
        ba_sb = persist.tile([E, 1], F32)
        b1sb = persist.tile([E, H], F32)
        b1bf = persist.tile([E, H], BF16)
        b1T = persist.tile([P, E], F32)
        expT = persist.tile([E, Bc], BF16)
        attn_be = persist.tile([P, Bc // P, E], F32)
        denomT = persist.tile([P, Bc // P], F32)
        recipT = persist.tile([P, Bc // P], F32)
        id_bf = persist.tile([P, P], BF16)
        rep_sel = persist.tile([E, E, P], BF16)
        junk = persist.tile([P, NT_SIZE], BF16)

        # ================= emission (program order matters per engine) ======
        nc.vector.memset(junk, 0.0)
        make_identity(nc, id_bf)

        # sync ring first in trace order: small fp32 weights (the gpsimd
        # casts below read these, so the DMAs must be traced first)
        nc.sync.dma_start(ba_sb[:], ba_ap[:, None])
        nc.sync.dma_start(b1sb[:], b1_ap[:])
        nc.sync.dma_start(W2sb[:], W2_ap.rearrange("e h o -> h e o"))
        nc.sync.dma_start(b2sb[:], b2_ap[:])

        # gpsimd SWDGE FIFO: z nt0 casts, setup, W1 stream (z nt1 slotted in)
        zcs = {}

        def z_cast_dma(blk):
            zcs[blk] = zc_pool.tile([P, D], BF16, tag="zc", name=f"zc{blk}")
            for h2 in range(2):
                nc.gpsimd.dma_start(
                    zcs[blk][:, h2 * D // 2:(h2 + 1) * D // 2],
                    z_ap[blk * P:(blk + 1) * P, h2 * D // 2:(h2 + 1) * D // 2])

        for blk in range(SUBS):
            z_cast_dma(blk)
        nc.gpsimd.memset(rep_sel, 0.0)
        nc.gpsimd.affine_select(
            out=rep_sel, in_=rep_sel,
            compare_op=ALU.not_equal, fill=1.0, base=0,
            pattern=[[-1, E], [0, P]], channel_multiplier=1,
        )

        def w1_load(e):
            # fully contiguous: partition p <- rows 16p..16p+15 of W1[e]
            nc.gpsimd.dma_start(
                w1j[e][:], W1_ap[e].rearrange("(p j) h -> p (j h)", p=P))

        # Wa in the same j-layout, one contiguous cast-DMA (1KB/partition)
        nc.gpsimd.dma_start(
            Wabf[:], Wa_ap.rearrange("(p j) e -> p (j e)", p=P))
        w1_load(0)
        w1_load(1)
        nc.gpsimd.tensor_copy(W2bf[:], W2sb[:])
        nc.gpsimd.tensor_copy(b2bf[:], b2sb[:])
        for e in range(2, 10):
            w1_load(e)
        for blk in range(SUBS, NBLK):   # z nt1, behind W1[9] in the FIFO
            z_cast_dma(blk)
        for e in range(10, E):
            w1_load(e)

        # DVE early casts
        nc.vector.tensor_copy(b1bf[:], b1sb[:])

        # ---- PE stream ----
        def warm(n):
            for _ in range(n):
                ps_j = psB.tile([P, NT_SIZE], F32, tag="ps_r", name="ps_warm")
                nc.tensor.matmul(ps_j[:], junk[:, :P], junk[:],
                                 start=True, stop=True)

        def pe_transpose_block(blk):
            # zTj[p, j, blk cols] = z[b, 16p+j]: strided column slices
            zr = zcs[blk][:].rearrange("b (p j) -> b j p", j=KO)
            for jp in range(KO // 2):
                ps_t = psD.tile([P, 2 * P], BF16, tag="ps_tr", name="ps_ztr")
                for k in range(2):
                    nc.tensor.transpose(
                        ps_t[:, k * P:(k + 1) * P], zr[:, 2 * jp + k, :], id_bf[:])
                dst = zTj[:, 2 * jp:2 * jp + 2, blk * P:(blk + 1) * P]
                if jp % 2 == 0:
                    nc.scalar.copy(dst, ps_t[:].rearrange("p (k b) -> p k b", k=2))
                else:
                    nc.vector.tensor_copy(
                        dst, ps_t[:].rearrange("p (k b) -> p k b", k=2))

        warm(16)
        for blk in range(SUBS):
            pe_transpose_block(blk)
            warm(6)

        ps_b1 = psD.tile([P, E], BF16, tag="ps_tr")
        nc.tensor.transpose(ps_b1[:], b1bf[:], id_bf[:E, :E])
        nc.scalar.copy(b1T[:], ps_b1[:])

        def logits_nt(nt):
            bs = slice(nt * NT_SIZE, (nt + 1) * NT_SIZE)
            ps_l = psD.tile([E, NT_SIZE], F32, tag="ps_tr", name="ps_l")
            for ko in range(KO):
                nc.tensor.matmul(
                    ps_l[:], Wabf[:, ko, :], zTj[:, ko, bs],
                    start=(ko == 0), stop=(ko == KO - 1))
            nc.scalar.activation(expT[:, bs], ps_l[:], AF.Exp, bias=ba_sb[:])

        def denom_nt(nt):
            for sub in range(SUBS):
                blk = nt * SUBS + sub
                ps_t = psD.tile([P, E], BF16, tag="ps_tr")
                nc.tensor.transpose(
                    ps_t[:], expT[:, blk * P:(blk + 1) * P], id_bf[:E, :E])
                nc.scalar.copy(attn_be[:, blk, :], ps_t[:])
            nts = slice(nt * SUBS, (nt + 1) * SUBS)
            nc.vector.reduce_sum(
                denomT[:, nts, None], attn_be[:, nts, :], axis=mybir.AxisListType.X)
            nc.vector.reciprocal(recipT[:, nts], denomT[:, nts])

        def finalize_tail(nt, res):
            for sub in range(SUBS):
                blk = nt * SUBS + sub
                ps_t2 = psD.tile([P, O], BF16, tag="ps_tr")
                nc.tensor.transpose(
                    ps_t2[:], res[:, sub * P:(sub + 1) * P], id_bf[:O, :O])
                outsb = outsb_pool.tile([P, O], F32)
                nc.scalar.activation(outsb[:], ps_t2[:], AF.Copy,
                                     scale=recipT[:, blk:blk + 1])
                nc.sync.dma_start(out_ap[blk * P:(blk + 1) * P, :], outsb[:])

        logits_nt(0)

        # ---- main loop ----
        pend_w2 = []
        pend_fin = None

        def flush_w2(keep, stop=False):
            while len(pend_w2) > keep:
                pe_, phm, po = pend_w2.pop(0)
                nc.tensor.matmul(po[:], W2bf[:, pe_, :], phm[:],
                                 start=False, stop=(stop and not pend_w2))

        for nt in range(NT):
            bs = slice(nt * NT_SIZE, (nt + 1) * NT_SIZE)
            ps_o = psC.tile([O, NT_SIZE], F32)
            nc.tensor.matmul(ps_o[:], b2bf[:], expT[:, bs], start=True, stop=False)
            for e in range(E):
                ps_h = psA.tile([P, NT_SIZE], F32)
                for j in range(KO):
                    nc.tensor.matmul(
                        ps_h[:], w1j[e][:, j, :], zTj[:, j, bs],
                        start=(j == 0), stop=(j == KO - 1))
                ps_r = psB.tile([P, NT_SIZE], F32, tag="ps_r")
                nc.tensor.matmul(ps_r[:], rep_sel[:, e, :], expT[:, bs],
                                 start=True, stop=True)
                flush_w2(1)
                if e == 2:
                    denom_nt(nt)
                if e == 13 and nt + 1 < NT:
                    logits_nt(nt + 1)
                if pend_fin is not None and e == 1:
                    finalize_tail(*pend_fin)
                    pend_fin = None
                t = t_pool.tile([P, NT_SIZE], F32)
                nc.scalar.activation(t[:], ps_h[:], AF.Relu, bias=b1T[:, e:e + 1])
                hm = hm_pool.tile([P, NT_SIZE], BF16)
                nc.vector.tensor_tensor(hm[:], t[:], ps_r[:], ALU.mult)
                pend_w2.append((e, hm, ps_o))
                # nt1 z prep: PE transposes, all before logits(nt1)@e13
                if nt == 0 and 8 <= e < 8 + SUBS:
                    pe_transpose_block(SUBS + (e - 8))
            flush_w2(0, stop=True)
            # read ps_o now (per sub-block, so the finalize chain pipelines):
            # the next nt's b2 matmul (start=True) reuses the same PSUM bank
            # and must come after these copies in program order
            res = res_pool.tile([O, NT_SIZE], BF16)
            for sub in range(SUBS):
                nc.vector.tensor_copy(res[:, sub * P:(sub + 1) * P],
                                      ps_o[:, sub * P:(sub + 1) * P])
            pend_fin = (nt, res)
        finalize_tail(*pend_fin)

    nc.compile()
    return nc


def ref_numpy(z, W1, b1, W2, b2, Wa, ba):
    B = z.shape[0]
    z = z.reshape(B, -1).astype(np.float64)
    logits = z @ Wa.astype(np.float64) + ba
    a = np.exp(logits - logits.max(axis=1, keepdims=True))
    a /= a.sum(axis=1, keepdims=True)
    h = np.maximum(np.einsum("bi,eih->beh", z, W1.astype(np.float64)) + b1, 0)
    o = np.einsum("beh,eho->beo", h, W2.astype(np.float64)) + b2
    return np.einsum("be,beo->bo", a, o).astype(np.float32)


# ---------------------------------------------------------------------------
# Harness entry point
# ---------------------------------------------------------------------------
N_CORES = 8
B_TOTAL = 8192
BC = B_TOTAL // N_CORES

_nc_cache = {}


def _get_nc():
    if "nc" not in _nc_cache:
        _nc_cache["nc"] = build_kernel(BC)
    return _nc_cache["nc"]


def kernel(z_i, W1, b1, W2, b2, Wa, ba):
    from concourse.bass_utils import run_bass_kernel_spmd

    z = np.ascontiguousarray(np.asarray(z_i, dtype=np.float32).reshape(B_TOTAL, D))
    W1 = np.ascontiguousarray(np.asarray(W1, dtype=np.float32))
    b1 = np.ascontiguousarray(np.asarray(b1, dtype=np.float32))
    W2 = np.ascontiguousarray(np.asarray(W2, dtype=np.float32))
    b2 = np.ascontiguousarray(np.asarray(b2, dtype=np.float32))
    Wa = np.ascontiguousarray(np.asarray(Wa, dtype=np.float32))
    ba = np.ascontiguousarray(np.asarray(ba, dtype=np.float32))

    nc = _get_nc()
    in_maps = [
        dict(z=z[c * BC:(c + 1) * BC], W1=W1, b1=b1, W2=W2, b2=b2, Wa=Wa, ba=ba)
        for c in range(N_CORES)
    ]
    res = run_bass_kernel_spmd(nc, in_maps, core_ids=list(range(N_CORES)))
    return np.concatenate([res.results[c]["out"] for c in range(N_CORES)], axis=0)
